# revision 24
# baseline (speedup 1.0000x reference)
"""Trainium2 Bass kernel for nn_MetaLearnerNetwork (3-layer GRU meta-learner).

Strategy (self-contained, hardcoded for B=128, T=1024, H=256, YD=30, L=3):

  * The reference scans the (time-reversed) sequence with a 3-layer GRU whose
    hidden state is zeroed wherever done==1.  A GRU forgets a zeroed initial
    state to fp32 noise within ~32 steps (measured 8.6e-7 at k=32 on this
    weight scale), so each row's 1024-step scan is split into 16 segments of
    64 committed steps, each preceded by a 32-step warmup re-processing the
    preceding tokens from h=0.  All 16 segments x 16 rows per core run in
    lockstep as 256 independent "lanes", turning the serial scan into 96 wide
    steps of [K,256]-moving matmuls per core.  8 cores are data-parallel over
    the 128 batch rows.  No collectives.

  * Layout: hidden on partitions (2 chunks of 128), lanes on the free dim.
    Gate psums accumulate W_ih @ x and W_hh @ h in one accumulation group;
    r/z biases ride the ACT sigmoid per-partition bias; n-gate biases ride
    fused scalar_tensor_tensor ops.  done-resets are applied as a per-lane
    keep-mask broadcast to 128 partitions with a rank-1 matmul.

  * fp32 everywhere; matmuls run as float32r (1 cycle/row at moving dim 256).
"""

import numpy as np

import concourse.bass as bass
import concourse.tile as tile
from concourse import bacc, mybir
from concourse.bass_utils import run_bass_kernel_spmd

F32 = mybir.dt.float32
F32R = mybir.dt.float32r
AF = mybir.ActivationFunctionType
OP = mybir.AluOpType

B, T, H, YD = 128, 1024, 256, 30
NCORES = 8
ROWS = B // NCORES            # 16 batch rows per core
SEG = 64                      # committed steps per segment
WARM = 24                     # warmup steps per segment (h error ~8e-6 by then)
NSEG = T // SEG               # 16 segments per row
STEPS = SEG + WARM            # 96 lockstep steps
LANES = ROWS * NSEG           # 256 lanes per core
CSTEPS = SEG                  # committed steps (i >= WARM)
CHUNK = 8                     # steps per streaming chunk
NCHUNK = STEPS // CHUNK       # 12
TOK = STEPS * LANES           # tokens per core (incl. warmup/pad)
CTOK = CHUNK * LANES          # tokens per chunk (2048)


def _rzcol(l, g, mc):
    # combined (b_ih+b_hh) bias column for layer l>=1, gate g (0=r,1=z), chunk mc
    return (l - 1) * 4 + g * 2 + mc


def _incol(l, mc):
    return 8 + (l - 1) * 2 + mc


def _hncol(l, mc):
    return 12 + l * 2 + mc


BFCY_COL = 18
NBIAS = 19


def build_nc(b_pi: float):
    nc = bacc.Bacc(None, target_bir_lowering=False)

    xf = nc.declare_dram_parameter("xf", [7, STEPS, LANES], F32R, isOutput=False)
    kp = nc.declare_dram_parameter("kp", [STEPS, LANES], F32, isOutput=False)
    yl = nc.declare_dram_parameter("yl", [31, TOK], F32, isOutput=False)
    y1l = nc.declare_dram_parameter("y1l", [31, TOK], F32, isOutput=False)
    wih0 = nc.declare_dram_parameter("wih0", [7, 768], F32R, isOutput=False)
    wih1 = nc.declare_dram_parameter("wih1", [2, 128, 768], F32R, isOutput=False)
    wih2 = nc.declare_dram_parameter("wih2", [2, 128, 768], F32R, isOutput=False)
    whh0 = nc.declare_dram_parameter("whh0", [2, 128, 768], F32R, isOutput=False)
    whh1 = nc.declare_dram_parameter("whh1", [2, 128, 768], F32R, isOutput=False)
    whh2 = nc.declare_dram_parameter("whh2", [2, 128, 768], F32R, isOutput=False)
    wfc = nc.declare_dram_parameter("wfc", [2, 128, 31], F32R, isOutput=False)
    we1 = nc.declare_dram_parameter("we1", [31, 16], F32, isOutput=False)
    we2 = nc.declare_dram_parameter("we2", [17, 1], F32, isOutput=False)
    bias = nc.declare_dram_parameter("bias", [128, NBIAS], F32, isOutput=False)
    zz = nc.declare_dram_parameter("zz", [2, LANES], F32R, isOutput=False)
    o1 = nc.declare_dram_parameter("o1", [CTOK], F32, isOutput=False)
    yo = nc.declare_dram_parameter("yo", [CSTEPS, 30, LANES], F32, isOutput=True)
    pi = nc.declare_dram_parameter("pi", [CSTEPS, LANES], F32, isOutput=True)

    wih_d = [wih0, wih1, wih2]
    whh_d = [whh0, whh1, whh2]

    with tile.TileContext(nc) as tc:
        with (
            tc.tile_pool(name="const", bufs=1) as const,
            tc.tile_pool(name="xpool", bufs=3) as xpool,
            tc.tile_pool(name="kpool", bufs=3) as kpool,
            tc.tile_pool(name="ypool", bufs=2) as ypool,
            tc.tile_pool(name="h1pool", bufs=2) as h1pool,
            tc.tile_pool(name="hpool", bufs=3) as hpool,
            tc.tile_pool(name="gpool", bufs=2) as gpool,
            tc.tile_pool(name="opool", bufs=3) as opool,
            tc.tile_pool(name="pp", bufs=1, space="PSUM") as pp,
        ):
            # ---- constants into SBUF ----
            wih0_sb = const.tile([7, 768], F32R)
            nc.sync.dma_start(out=wih0_sb, in_=wih0[:])
            wih_sb = [wih0_sb]
            for l in (1, 2):
                w = const.tile([128, 2, 768], F32R, name=f"wih{l}_sb")
                for kc in range(2):
                    nc.sync.dma_start(out=w[:, kc, :], in_=wih_d[l][kc])
                wih_sb.append(w)
            whh_sb = []
            for l in range(3):
                w = const.tile([128, 2, 768], F32R, name=f"whh{l}_sb")
                for kc in range(2):
                    nc.sync.dma_start(out=w[:, kc, :], in_=whh_d[l][kc])
                whh_sb.append(w)
            wfc_sb = const.tile([128, 2, 31], F32R)
            for kc in range(2):
                nc.sync.dma_start(out=wfc_sb[:, kc, :], in_=wfc[kc])
            we1_sb = const.tile([31, 16], F32)
            nc.sync.dma_start(out=we1_sb, in_=we1[:])
            we2_sb = const.tile([17, 1], F32)
            nc.sync.dma_start(out=we2_sb, in_=we2[:])
            bias_sb = const.tile([128, NBIAS], F32)
            nc.sync.dma_start(out=bias_sb, in_=bias[:])
            ones_sb = const.tile([1, 128], F32)
            nc.vector.memset(ones_sb, 1.0)

            # ---- persistent hidden state (zero-init) ----
            h_prev = []
            for l in range(3):
                h0 = hpool.tile([128, 2, LANES], F32R, name=f"h{l}", tag=f"h{l}")
                zsrc = zz[:]
                zb = bass.AP(tensor=zsrc.tensor, offset=zsrc.offset,
                             ap=[[0, 128]] + [list(d) for d in zsrc.ap])
                nc.sync.dma_start(out=h0, in_=zb)
                h_prev.append(h0)

            def embed_chunk(c, xc, xrow, src):
                yc = ypool.tile([31, CTOK], F32, name="yc", tag="yc")
                nc.sync.dma_start(out=yc, in_=src[:, c * CTOK:(c + 1) * CTOK])
                h1 = h1pool.tile([17, CTOK], F32, name="h1", tag="h1")
                nc.sync.dma_start(out=h1[16:17, :], in_=o1[:])  # e2 bias row
                fi_t = h1pool.tile([1, CTOK], F32R, name="fi_t", tag="fi_t")
                for q in range(CTOK // 512):
                    sl = slice(q * 512, (q + 1) * 512)
                    pe1 = pp.tile([16, 512], F32, name="pe1", tag="ps_misc", bufs=2)
                    nc.tensor.matmul(
                        pe1[:], we1_sb[:], yc[:, sl]
                    )
                    nc.vector.tensor_scalar_max(h1[0:16, sl], pe1[:], 0.0)
                    pe2 = pp.tile([1, 512], F32, name="pe2", tag="ps_misc", bufs=2)
                    nc.tensor.matmul(
                        pe2[:], we2_sb[:], h1[:, sl]
                    )
                    nc.scalar.activation(fi_t[:, sl], pe2[:], AF.Sigmoid)
                # engines can't write at a partition offset; DMA the feature
                # row into place instead
                nc.sync.dma_start(
                    out=xc[xrow:xrow + 1, :, :],
                    in_=fi_t[:].rearrange("p (a b) -> p a b", a=CHUNK),
                )

            def cell(l, k, x_i, kb_i, lin):
                """One GRU cell: layer l at lockstep step k.
                x_i: [7,LANES] (l==0) ; lin: hm of layer l-1 at step k (l>0)
                kb_i: [128, LANES] keep-mask (broadcast rows), applied to the
                hidden carried into step k+1.  Returns unmasked hm."""
                pr = pp.tile([128, 512], F32, name="pr", tag="ps_r", bufs=2)
                pz = pp.tile([128, 512], F32, name="pz", tag="ps_z", bufs=2)
                pgin = pp.tile([128, 512], F32, name="pgin", tag="ps_gin", bufs=1)
                pghn = pp.tile([128, 512], F32, name="pghn", tag="ps_ghn", bufs=1)
                wi, wh = wih_sb[l], whh_sb[l]
                # r and z accumulate gi + gh in one psum group
                for g, ps in ((0, pr), (1, pz)):
                    for mc in range(2):
                        dst = ps[:, mc * LANES:(mc + 1) * LANES]
                        lo = g * 256 + mc * 128
                        if l == 0:
                            nc.tensor.matmul(
                                dst, wih0_sb[:, lo:lo + 128],
                                x_i, start=True, stop=False,
                            )
                        else:
                            for kc in range(2):
                                nc.tensor.matmul(
                                    dst, wi[:, kc, lo:lo + 128],
                                    lin[:, kc, :],
                                    start=(kc == 0), stop=False,
                                )
                        for kc in range(2):
                            nc.tensor.matmul(
                                dst, wh[:, kc, lo:lo + 128],
                                h_prev[l][:, kc, :],
                                start=False, stop=(kc == 1),
                            )
                # n gate: gi and gh kept separate
                for mc in range(2):
                    lo = 512 + mc * 128
                    dst = pgin[:, mc * LANES:(mc + 1) * LANES]
                    if l == 0:
                        nc.tensor.matmul(
                            dst, wih0_sb[:, lo:lo + 128],
                            x_i, start=True, stop=True,
                        )
                    else:
                        for kc in range(2):
                            nc.tensor.matmul(
                                dst, wi[:, kc, lo:lo + 128],
                                lin[:, kc, :],
                                start=(kc == 0), stop=(kc == 1),
                            )
                    dst = pghn[:, mc * LANES:(mc + 1) * LANES]
                    for kc in range(2):
                        nc.tensor.matmul(
                            dst, wh[:, kc, lo:lo + 128],
                            h_prev[l][:, kc, :],
                            start=(kc == 0), stop=(kc == 1),
                        )

                # ---- gate elementwise; critical chain: r -> t -> s -> tanh ----
                r_t = gpool.tile([128, 512], F32, name="r_t", tag="r_t")
                z_t = gpool.tile([128, 512], F32, name="z_t", tag="z_t")
                for mc in range(2):
                    sl = slice(mc * LANES, (mc + 1) * LANES)
                    b = bias_sb[:, _rzcol(l, 0, mc):_rzcol(l, 0, mc) + 1] \
                        if l > 0 else 0.0
                    nc.scalar.activation(r_t[:, sl], pr[:, sl], AF.Sigmoid,
                                         bias=b)
                t_t = gpool.tile([128, 512], F32, name="t_t", tag="t_t")
                s_t = gpool.tile([128, 512], F32, name="s_t", tag="s_t")
                for mc in range(2):
                    sl = slice(mc * LANES, (mc + 1) * LANES)
                    bhn = bias_sb[:, _hncol(l, mc):_hncol(l, mc) + 1]
                    nc.vector.scalar_tensor_tensor(
                        t_t[:, sl], pghn[:, sl], bhn, r_t[:, sl],
                        OP.add, OP.mult,
                    )
                    bin_ = bias_sb[:, _incol(l, mc):_incol(l, mc) + 1] \
                        if l > 0 else 0.0
                    nc.vector.scalar_tensor_tensor(
                        s_t[:, sl], pgin[:, sl], bin_, t_t[:, sl],
                        OP.add, OP.add,
                    )
                n_t = gpool.tile([128, 512], F32, name="n_t", tag="n_t")
                nc.scalar.activation(n_t[:], s_t[:], AF.Tanh)
                # z is off the critical chain; emit after tanh
                for mc in range(2):
                    sl = slice(mc * LANES, (mc + 1) * LANES)
                    b = bias_sb[:, _rzcol(l, 1, mc):_rzcol(l, 1, mc) + 1] \
                        if l > 0 else 0.0
                    nc.scalar.activation(z_t[:, sl], pz[:, sl], AF.Sigmoid,
                                         bias=b)
                hp = h_prev[l].rearrange("p a b -> p (a b)")
                # hm = n*(1-z) + z*h ; a and q are off-chain
                a_t = gpool.tile([128, 512], F32, name="a_t", tag="a_t")
                nc.gpsimd.tensor_scalar(a_t[:], z_t[:], -1.0, 1.0,
                                        OP.mult, OP.add)
                q_t = gpool.tile([128, 512], F32, name="q_t", tag="q_t")
                nc.gpsimd.tensor_mul(q_t[:], z_t[:], hp)
                v_t = gpool.tile([128, 512], F32, name="v_t", tag="v_t")
                nc.gpsimd.tensor_mul(v_t[:], n_t[:], a_t[:])
                hm = gpool.tile([128, 2, LANES], F32R, name=f"hm{l}",
                                tag=f"hm{l}")
                hmf = hm.rearrange("p a b -> p (a b)")
                nc.gpsimd.tensor_add(hmf, v_t[:], q_t[:])
                hn = hpool.tile([128, 2, LANES], F32R, name=f"h{l}",
                                tag=f"h{l}")
                # keep-mask, free-dim repeated across the two hidden chunks
                kb_rep = bass.AP(tensor=kb_i.tensor, offset=kb_i.offset,
                                 ap=[list(kb_i.ap[0]), [0, 2]]
                                 + [list(d) for d in kb_i.ap[1:]])
                nc.gpsimd.tensor_mul(hn, hm, kb_rep)
                h_prev[l] = hn
                return hm

            def heads(i, hm2):
                ph = pp.tile([30, LANES], F32, name="ph", tag="ps_misc", bufs=2)
                php = pp.tile([1, LANES], F32, name="php", tag="ps_misc", bufs=2)
                for kc in range(2):
                    nc.tensor.matmul(
                        ph[:], wfc_sb[:, kc, 0:30],
                        hm2[:, kc, :],
                        start=(kc == 0), stop=(kc == 1),
                    )
                    nc.tensor.matmul(
                        php[:], wfc_sb[:, kc, 30:31],
                        hm2[:, kc, :],
                        start=(kc == 0), stop=(kc == 1),
                    )
                yo_t = opool.tile([30, LANES], F32, name="yo_t", tag="yo_t")
                nc.scalar.activation(yo_t[:], ph[:], AF.Sigmoid,
                                     bias=bias_sb[0:30, BFCY_COL:BFCY_COL + 1])
                pi_t = opool.tile([1, LANES], F32, name="pi_t", tag="pi_t")
                nc.vector.tensor_scalar_add(pi_t[:], php[:], b_pi)
                nc.sync.dma_start(out=yo[i - WARM], in_=yo_t[:])
                nc.sync.dma_start(out=pi[i - WARM], in_=pi_t[:])

            # ---- main loop: wavefront emission ----
            # wave w emits layer0@k=w, layer1@k=w-1, layer2@k=w-2 so every
            # engine's in-order queue interleaves three independent chains.
            LEAD = 1
            xcs, kbs = {}, {}
            next_chunk = [0]

            def load_chunk(c):
                xc = xpool.tile([7, CHUNK, LANES], F32R, name="xc", tag="xc")
                nc.sync.dma_start(out=xc, in_=xf[:, c * CHUNK:(c + 1) * CHUNK, :])
                kb = kpool.tile([128, CHUNK, LANES], F32, name="kb", tag="kb")
                src_ap = kp[c * CHUNK:(c + 1) * CHUNK, :]
                bsrc = bass.AP(tensor=src_ap.tensor, offset=src_ap.offset,
                               ap=[[0, 128]] + [list(d) for d in src_ap.ap])
                nc.sync.dma_start(out=kb, in_=bsrc)
                embed_chunk(c, xc, 4, yl)
                embed_chunk(c, xc, 5, y1l)
                xcs[c], kbs[c] = xc, kb

            hm_wave = {}
            for w in range(STEPS + 2):
                want = min(w // CHUNK + 1 + LEAD, NCHUNK)
                while next_chunk[0] < want:
                    load_chunk(next_chunk[0])
                    next_chunk[0] += 1
                new_hm = {}
                for l in range(3):
                    k = w - l
                    if not (0 <= k < STEPS):
                        continue
                    c, il = divmod(k, CHUNK)
                    x_i = xcs[c][:, il, :] if l == 0 else None
                    kb_i = kbs[c][:, il, :]
                    lin = hm_wave.get(l - 1)
                    hm = cell(l, k, x_i, kb_i, lin)
                    new_hm[l] = hm
                    if l == 2 and k >= WARM:
                        heads(k, hm)
                hm_wave = new_hm
                # release chunks no longer referenced (layer2 done past them)
                for c in [c for c in xcs if (c + 1) * CHUNK + 2 <= w + 1]:
                    xcs.pop(c), kbs.pop(c)

    nc.compile()
    return nc


def prepare_inputs(inp):
    """Host-side prep: returns (b_pi, in_maps) for the 8 cores."""
    f32 = np.float32
    rew = np.asarray(inp["rew"], f32)
    done = np.asarray(inp["done"], f32)
    gamma = float(np.asarray(inp["gamma"]).reshape(-1)[0])
    prob = np.asarray(inp["prob"], f32)
    y = np.asarray(inp["y"], f32)
    y1 = np.asarray(inp["y1"], f32)

    j_idx = np.arange(NSEG)[:, None]          # [16,1]
    i_idx = np.arange(STEPS)[None, :]         # [1,96]
    s_grid = SEG * j_idx - WARM + i_idx       # [16,96]
    valid = (s_grid >= 0)
    t_grid = np.clip(T - 1 - s_grid, 0, T - 1)  # [16,96]

    vmask = valid.astype(f32)                 # [16,96]

    def gather_feat(arr):
        # arr [B, T] -> [B, NSEG, STEPS] gathered, zeroed at pads
        return arr[:, t_grid] * vmask[None]

    rew_g = gather_feat(rew)
    done_g = gather_feat(done)
    prob_g = gather_feat(prob)

    # reset mask: done at that token; forced at the first real token of j=0
    reset = done_g.copy()                     # [B,16,96]
    reset[:, 0, WARM] = 1.0
    keep = 1.0 - reset                        # [B,16,96]
    # mask applied at END of step i must be the keep of step i+1
    keep_next = np.ones_like(keep)
    keep_next[:, :, :-1] = keep[:, :, 1:]

    # embed inputs gathered to lane order  [B,16,96,30]
    y_g = y[:, t_grid, :] * vmask[None, :, :, None]
    y1_g = y1[:, t_grid, :] * vmask[None, :, :, None]

    wih0_aug = np.zeros((7, 768), f32)
    wih0_aug[0:6] = np.asarray(inp["w_ih0"], f32).T
    brz = (np.asarray(inp["b_ih0"], f32) + np.asarray(inp["b_hh0"], f32))
    wih0_aug[6, 0:512] = brz[0:512]
    wih0_aug[6, 512:768] = np.asarray(inp["b_ih0"], f32)[512:768]

    def wT3(name):
        return np.ascontiguousarray(
            np.asarray(inp[name], f32).T.reshape(2, 128, 768))

    wfc_np = np.concatenate(
        [np.asarray(inp["w_fc_y"], f32), np.asarray(inp["w_fc_pi"], f32)], 0
    ).T.reshape(2, 128, 31)
    wfc_np = np.ascontiguousarray(wfc_np)

    we1_aug = np.zeros((31, 16), f32)
    we1_aug[0:30] = np.asarray(inp["w_e1"], f32).T
    we1_aug[30] = np.asarray(inp["b_e1"], f32)
    we2_aug = np.zeros((17, 1), f32)
    we2_aug[0:16] = np.asarray(inp["w_e2"], f32).T
    we2_aug[16, 0] = float(np.asarray(inp["b_e2"]).reshape(-1)[0])

    bias_np = np.zeros((128, NBIAS), f32)
    for l in (1, 2):
        comb = np.asarray(inp[f"b_ih{l}"], f32) + np.asarray(inp[f"b_hh{l}"], f32)
        for g in range(2):
            for mc in range(2):
                lo = g * 256 + mc * 128
                bias_np[:, _rzcol(l, g, mc)] = comb[lo:lo + 128]
        for mc in range(2):
            bias_np[:, _incol(l, mc)] = \
                np.asarray(inp[f"b_ih{l}"], f32)[512 + mc * 128:640 + mc * 128]
    for l in range(3):
        for mc in range(2):
            bias_np[:, _hncol(l, mc)] = \
                np.asarray(inp[f"b_hh{l}"], f32)[512 + mc * 128:640 + mc * 128]
    bias_np[0:30, BFCY_COL] = np.asarray(inp["b_fc_y"], f32)

    b_pi = float(np.asarray(inp["b_fc_pi"]).reshape(-1)[0])

    shared = dict(
        wih0=wih0_aug, wih1=wT3("w_ih1"), wih2=wT3("w_ih2"),
        whh0=wT3("w_hh0"), whh1=wT3("w_hh1"), whh2=wT3("w_hh2"),
        wfc=wfc_np, we1=we1_aug, we2=we2_aug, bias=bias_np,
        zz=np.zeros((2, LANES), f32), o1=np.ones((CTOK,), f32),
    )

    in_maps = []
    for c in range(NCORES):
        R = slice(c * ROWS, (c + 1) * ROWS)
        # xf [7, STEPS, LANES]; lane = r*NSEG + j
        xf_np = np.zeros((7, STEPS, LANES), f32)

        def lanes(a):  # [16r, 16j, 96i] -> [96i, 256lane]
            return a.transpose(2, 0, 1).reshape(STEPS, LANES)

        xf_np[0] = lanes(rew_g[R])
        xf_np[1] = lanes(done_g[R])
        xf_np[2] = gamma * np.tile(vmask.T.reshape(STEPS, 1, NSEG),
                                   (1, ROWS, 1)).reshape(STEPS, LANES)
        xf_np[3] = lanes(prob_g[R])
        xf_np[6] = 1.0
        kp_np = lanes(keep_next[R])           # [96, 256]
        # y lanes [31, TOK]: token index = i*LANES + lane
        yl_np = np.ones((31, TOK), f32)
        yl_np[0:30] = y_g[R].transpose(3, 2, 0, 1).reshape(30, TOK)
        y1l_np = np.ones((31, TOK), f32)
        y1l_np[0:30] = y1_g[R].transpose(3, 2, 0, 1).reshape(30, TOK)
        in_maps.append(dict(shared, xf=xf_np, kp=kp_np, yl=yl_np, y1l=y1l_np))
    return b_pi, in_maps


def assemble_outputs(results):
    pi_proc = np.zeros((B, T), np.float32)
    yo_proc = np.zeros((B, T, YD), np.float32)
    for c in range(NCORES):
        R = slice(c * ROWS, (c + 1) * ROWS)
        yo_c = results[c]["yo"]               # [64, 30, 256]
        pi_c = results[c]["pi"]               # [64, 256]
        yo_proc[R] = yo_c.reshape(CSTEPS, YD, ROWS, NSEG) \
            .transpose(2, 3, 0, 1).reshape(ROWS, T, YD)
        pi_proc[R] = pi_c.reshape(CSTEPS, ROWS, NSEG) \
            .transpose(1, 2, 0).reshape(ROWS, T)
    # processing order s -> original time t = T-1-s
    return np.ascontiguousarray(pi_proc[:, ::-1]), \
        np.ascontiguousarray(yo_proc[:, ::-1])


def kernel(**inputs):
    b_pi, in_maps = prepare_inputs(inputs)
    nc = build_nc(b_pi)
    res = run_bass_kernel_spmd(nc, in_maps, list(range(NCORES)))
    return assemble_outputs(res.results)


# revision 26
# speedup vs baseline: 1.2114x; 1.2114x over previous
"""Trainium2 Bass kernel for nn_MetaLearnerNetwork (3-layer GRU meta-learner).

Strategy (self-contained, hardcoded for B=128, T=1024, H=256, YD=30, L=3):

  * The reference scans the (time-reversed) sequence with a 3-layer GRU whose
    hidden state is zeroed wherever done==1.  A GRU forgets a zeroed initial
    state to fp32 noise within ~32 steps (measured 8.6e-7 at k=32 on this
    weight scale), so each row's 1024-step scan is split into 16 segments of
    64 committed steps, each preceded by a 32-step warmup re-processing the
    preceding tokens from h=0.  All 16 segments x 16 rows per core run in
    lockstep as 256 independent "lanes", turning the serial scan into 96 wide
    steps of [K,256]-moving matmuls per core.  8 cores are data-parallel over
    the 128 batch rows.  No collectives.

  * Layout: hidden on partitions (2 chunks of 128), lanes on the free dim.
    Gate psums accumulate W_ih @ x and W_hh @ h in one accumulation group;
    r/z biases ride the ACT sigmoid per-partition bias; n-gate biases ride
    fused scalar_tensor_tensor ops.  done-resets are applied as a per-lane
    keep-mask broadcast to 128 partitions with a rank-1 matmul.

  * fp32 everywhere; matmuls run as float32r (1 cycle/row at moving dim 256).
"""

import numpy as np

import concourse.bass as bass
import concourse.tile as tile
from concourse import bacc, mybir
from concourse.bass_utils import run_bass_kernel_spmd

F32 = mybir.dt.float32
F32R = mybir.dt.float32r
F16 = mybir.dt.float16
AF = mybir.ActivationFunctionType
OP = mybir.AluOpType

B, T, H, YD = 128, 1024, 256, 30
NCORES = 8
ROWS = B // NCORES            # 16 batch rows per core
SEG = 64                      # committed steps per segment
WARM = 24                     # warmup steps per segment (h error ~8e-6 by then)
NSEG = T // SEG               # 16 segments per row
STEPS = SEG + WARM            # 96 lockstep steps
LANES = ROWS * NSEG           # 256 lanes per core
CSTEPS = SEG                  # committed steps (i >= WARM)
CHUNK = 8                     # steps per streaming chunk
NCHUNK = STEPS // CHUNK       # 12
TOK = STEPS * LANES           # tokens per core (incl. warmup/pad)
CTOK = CHUNK * LANES          # tokens per chunk (2048)


def _rzcol(l, g, mc):
    # combined (b_ih+b_hh) bias column for layer l>=1, gate g (0=r,1=z), chunk mc
    return (l - 1) * 4 + g * 2 + mc


def _incol(l, mc):
    return 8 + (l - 1) * 2 + mc


def _hncol(l, mc):
    return 12 + l * 2 + mc


BFCY_COL = 18
NBIAS = 19


def build_nc(b_pi: float):
    nc = bacc.Bacc(None, target_bir_lowering=False)

    xf = nc.declare_dram_parameter("xf", [7, STEPS, LANES], F16, isOutput=False)
    kp = nc.declare_dram_parameter("kp", [STEPS, LANES], F16, isOutput=False)
    yl = nc.declare_dram_parameter("yl", [31, TOK], F32, isOutput=False)
    y1l = nc.declare_dram_parameter("y1l", [31, TOK], F32, isOutput=False)
    wih0 = nc.declare_dram_parameter("wih0", [7, 768], F16, isOutput=False)
    wih1 = nc.declare_dram_parameter("wih1", [2, 128, 768], F16, isOutput=False)
    wih2 = nc.declare_dram_parameter("wih2", [2, 128, 768], F16, isOutput=False)
    whh0 = nc.declare_dram_parameter("whh0", [2, 128, 768], F16, isOutput=False)
    whh1 = nc.declare_dram_parameter("whh1", [2, 128, 768], F16, isOutput=False)
    whh2 = nc.declare_dram_parameter("whh2", [2, 128, 768], F16, isOutput=False)
    wfc = nc.declare_dram_parameter("wfc", [2, 128, 31], F16, isOutput=False)
    we1 = nc.declare_dram_parameter("we1", [31, 16], F32, isOutput=False)
    we2 = nc.declare_dram_parameter("we2", [17, 1], F32, isOutput=False)
    bias = nc.declare_dram_parameter("bias", [128, NBIAS], F32, isOutput=False)
    zz = nc.declare_dram_parameter("zz", [2, LANES], F16, isOutput=False)
    o1 = nc.declare_dram_parameter("o1", [CTOK], F32, isOutput=False)
    yo = nc.declare_dram_parameter("yo", [CSTEPS, 30, LANES], F32, isOutput=True)
    pi = nc.declare_dram_parameter("pi", [CSTEPS, LANES], F32, isOutput=True)

    wih_d = [wih0, wih1, wih2]
    whh_d = [whh0, whh1, whh2]

    with tile.TileContext(nc) as tc:
        with (
            tc.tile_pool(name="const", bufs=1) as const,
            tc.tile_pool(name="xpool", bufs=3) as xpool,
            tc.tile_pool(name="kpool", bufs=3) as kpool,
            tc.tile_pool(name="ypool", bufs=2) as ypool,
            tc.tile_pool(name="h1pool", bufs=2) as h1pool,
            tc.tile_pool(name="hpool", bufs=3) as hpool,
            tc.tile_pool(name="gpool", bufs=2) as gpool,
            tc.tile_pool(name="opool", bufs=3) as opool,
            tc.tile_pool(name="pp", bufs=1, space="PSUM") as pp,
        ):
            # ---- constants into SBUF ----
            wih0_sb = const.tile([7, 768], F16)
            nc.sync.dma_start(out=wih0_sb, in_=wih0[:])
            wih_sb = [wih0_sb]
            for l in (1, 2):
                w = const.tile([128, 2, 768], F16, name=f"wih{l}_sb")
                for kc in range(2):
                    nc.sync.dma_start(out=w[:, kc, :], in_=wih_d[l][kc])
                wih_sb.append(w)
            whh_sb = []
            for l in range(3):
                w = const.tile([128, 2, 768], F16, name=f"whh{l}_sb")
                for kc in range(2):
                    nc.sync.dma_start(out=w[:, kc, :], in_=whh_d[l][kc])
                whh_sb.append(w)
            wfc_sb = const.tile([128, 2, 31], F16)
            for kc in range(2):
                nc.sync.dma_start(out=wfc_sb[:, kc, :], in_=wfc[kc])
            we1_sb = const.tile([31, 16], F32)
            nc.sync.dma_start(out=we1_sb, in_=we1[:])
            we2_sb = const.tile([17, 1], F32)
            nc.sync.dma_start(out=we2_sb, in_=we2[:])
            bias_sb = const.tile([128, NBIAS], F32)
            nc.sync.dma_start(out=bias_sb, in_=bias[:])
            ones_sb = const.tile([1, 128], F32)
            nc.vector.memset(ones_sb, 1.0)

            # ---- persistent hidden state (zero-init) ----
            h_prev = []
            for l in range(3):
                h0 = hpool.tile([128, 2, LANES], F16, name=f"h{l}", tag=f"h{l}")
                zsrc = zz[:]
                zb = bass.AP(tensor=zsrc.tensor, offset=zsrc.offset,
                             ap=[[0, 128]] + [list(d) for d in zsrc.ap])
                nc.sync.dma_start(out=h0, in_=zb)
                h_prev.append(h0)

            def embed_chunk(c, xc, xrow, src):
                yc = ypool.tile([31, CTOK], F32, name="yc", tag="yc")
                nc.sync.dma_start(out=yc, in_=src[:, c * CTOK:(c + 1) * CTOK])
                h1 = h1pool.tile([17, CTOK], F32, name="h1", tag="h1")
                nc.sync.dma_start(out=h1[16:17, :], in_=o1[:])  # e2 bias row
                fi_t = h1pool.tile([1, CTOK], F16, name="fi_t", tag="fi_t")
                for q in range(CTOK // 512):
                    sl = slice(q * 512, (q + 1) * 512)
                    pe1 = pp.tile([16, 512], F32, name="pe1", tag="ps_misc", bufs=2)
                    nc.tensor.matmul(
                        pe1[:], we1_sb[:], yc[:, sl]
                    )
                    nc.vector.tensor_scalar_max(h1[0:16, sl], pe1[:], 0.0)
                    pe2 = pp.tile([1, 512], F32, name="pe2", tag="ps_misc", bufs=2)
                    nc.tensor.matmul(
                        pe2[:], we2_sb[:], h1[:, sl]
                    )
                    nc.scalar.activation(fi_t[:, sl], pe2[:], AF.Sigmoid)
                # engines can't write at a partition offset; DMA the feature
                # row into place instead
                nc.sync.dma_start(
                    out=xc[xrow:xrow + 1, :, :],
                    in_=fi_t[:].rearrange("p (a b) -> p a b", a=CHUNK),
                )

            def cell(l, k, x_i, kb_i, lin):
                """One GRU cell: layer l at lockstep step k.
                x_i: [7,LANES] (l==0) ; lin: hm of layer l-1 at step k (l>0)
                kb_i: [128, LANES] keep-mask (broadcast rows), applied to the
                hidden carried into step k+1.  Returns unmasked hm."""
                pr = pp.tile([128, 512], F32, name="pr", tag="ps_r", bufs=2)
                pz = pp.tile([128, 512], F32, name="pz", tag="ps_z", bufs=2)
                pgin = pp.tile([128, 512], F32, name="pgin", tag="ps_gin", bufs=1)
                pghn = pp.tile([128, 512], F32, name="pghn", tag="ps_ghn", bufs=1)
                wi, wh = wih_sb[l], whh_sb[l]
                # r and z accumulate gi + gh in one psum group
                for g, ps in ((0, pr), (1, pz)):
                    for mc in range(2):
                        dst = ps[:, mc * LANES:(mc + 1) * LANES]
                        lo = g * 256 + mc * 128
                        if l == 0:
                            nc.tensor.matmul(
                                dst, wih0_sb[:, lo:lo + 128],
                                x_i, start=True, stop=False,
                            )
                        else:
                            for kc in range(2):
                                nc.tensor.matmul(
                                    dst, wi[:, kc, lo:lo + 128],
                                    lin[:, kc, :],
                                    start=(kc == 0), stop=False,
                                )
                        for kc in range(2):
                            nc.tensor.matmul(
                                dst, wh[:, kc, lo:lo + 128],
                                h_prev[l][:, kc, :],
                                start=False, stop=(kc == 1),
                            )
                # n gate: gi and gh kept separate
                for mc in range(2):
                    lo = 512 + mc * 128
                    dst = pgin[:, mc * LANES:(mc + 1) * LANES]
                    if l == 0:
                        nc.tensor.matmul(
                            dst, wih0_sb[:, lo:lo + 128],
                            x_i, start=True, stop=True,
                        )
                    else:
                        for kc in range(2):
                            nc.tensor.matmul(
                                dst, wi[:, kc, lo:lo + 128],
                                lin[:, kc, :],
                                start=(kc == 0), stop=(kc == 1),
                            )
                    dst = pghn[:, mc * LANES:(mc + 1) * LANES]
                    for kc in range(2):
                        nc.tensor.matmul(
                            dst, wh[:, kc, lo:lo + 128],
                            h_prev[l][:, kc, :],
                            start=(kc == 0), stop=(kc == 1),
                        )

                # ---- gate elementwise; critical chain: r -> t -> s -> tanh ----
                r_t = gpool.tile([128, 512], F16, name="r_t", tag="r_t")
                z_t = gpool.tile([128, 512], F16, name="z_t", tag="z_t")
                for mc in range(2):
                    sl = slice(mc * LANES, (mc + 1) * LANES)
                    b = bias_sb[:, _rzcol(l, 0, mc):_rzcol(l, 0, mc) + 1] \
                        if l > 0 else 0.0
                    nc.scalar.activation(r_t[:, sl], pr[:, sl], AF.Sigmoid,
                                         bias=b)
                t_t = gpool.tile([128, 512], F16, name="t_t", tag="t_t")
                s_t = gpool.tile([128, 512], F16, name="s_t", tag="s_t")
                for mc in range(2):
                    sl = slice(mc * LANES, (mc + 1) * LANES)
                    bhn = bias_sb[:, _hncol(l, mc):_hncol(l, mc) + 1]
                    nc.vector.scalar_tensor_tensor(
                        t_t[:, sl], pghn[:, sl], bhn, r_t[:, sl],
                        OP.add, OP.mult,
                    )
                    bin_ = bias_sb[:, _incol(l, mc):_incol(l, mc) + 1] \
                        if l > 0 else 0.0
                    nc.vector.scalar_tensor_tensor(
                        s_t[:, sl], pgin[:, sl], bin_, t_t[:, sl],
                        OP.add, OP.add,
                    )
                n_t = gpool.tile([128, 512], F16, name="n_t", tag="n_t")
                nc.scalar.activation(n_t[:], s_t[:], AF.Tanh)
                # z is off the critical chain; emit after tanh
                for mc in range(2):
                    sl = slice(mc * LANES, (mc + 1) * LANES)
                    b = bias_sb[:, _rzcol(l, 1, mc):_rzcol(l, 1, mc) + 1] \
                        if l > 0 else 0.0
                    nc.scalar.activation(z_t[:, sl], pz[:, sl], AF.Sigmoid,
                                         bias=b)
                hp = h_prev[l].rearrange("p a b -> p (a b)")
                # hm = n + z*(h - n); d,e on Pool, rest on DVE (fp16 2x mode)
                d_t = gpool.tile([128, 512], F16, name="d_t", tag="d_t")
                nc.gpsimd.tensor_sub(d_t[:], hp, n_t[:])
                e_t = gpool.tile([128, 512], F16, name="e_t", tag="e_t")
                nc.gpsimd.tensor_mul(e_t[:], z_t[:], d_t[:])
                hm = gpool.tile([128, 2, LANES], F16, name=f"hm{l}",
                                tag=f"hm{l}")
                hmf = hm.rearrange("p a b -> p (a b)")
                nc.vector.tensor_add(hmf, n_t[:], e_t[:])
                hn = hpool.tile([128, 2, LANES], F16, name=f"h{l}",
                                tag=f"h{l}")
                # keep-mask, free-dim repeated across the two hidden chunks
                kb_rep = bass.AP(tensor=kb_i.tensor, offset=kb_i.offset,
                                 ap=[list(kb_i.ap[0]), [0, 2]]
                                 + [list(d) for d in kb_i.ap[1:]])
                nc.gpsimd.tensor_mul(hn, hm, kb_rep)
                h_prev[l] = hn
                return hm

            def heads(i, hm2):
                ph = pp.tile([30, LANES], F32, name="ph", tag="ps_misc", bufs=2)
                php = pp.tile([1, LANES], F32, name="php", tag="ps_misc", bufs=2)
                for kc in range(2):
                    nc.tensor.matmul(
                        ph[:], wfc_sb[:, kc, 0:30],
                        hm2[:, kc, :],
                        start=(kc == 0), stop=(kc == 1),
                    )
                    nc.tensor.matmul(
                        php[:], wfc_sb[:, kc, 30:31],
                        hm2[:, kc, :],
                        start=(kc == 0), stop=(kc == 1),
                    )
                yo_t = opool.tile([30, LANES], F32, name="yo_t", tag="yo_t")
                nc.scalar.activation(yo_t[:], ph[:], AF.Sigmoid,
                                     bias=bias_sb[0:30, BFCY_COL:BFCY_COL + 1])
                pi_t = opool.tile([1, LANES], F32, name="pi_t", tag="pi_t")
                nc.vector.tensor_scalar_add(pi_t[:], php[:], b_pi)
                nc.sync.dma_start(out=yo[i - WARM], in_=yo_t[:])
                nc.sync.dma_start(out=pi[i - WARM], in_=pi_t[:])

            # ---- main loop: wavefront emission ----
            # wave w emits layer0@k=w, layer1@k=w-1, layer2@k=w-2 so every
            # engine's in-order queue interleaves three independent chains.
            LEAD = 1
            xcs, kbs = {}, {}
            next_chunk = [0]

            def load_chunk(c):
                xc = xpool.tile([7, CHUNK, LANES], F16, name="xc", tag="xc")
                nc.sync.dma_start(out=xc, in_=xf[:, c * CHUNK:(c + 1) * CHUNK, :])
                kb = kpool.tile([128, CHUNK, LANES], F16, name="kb", tag="kb")
                src_ap = kp[c * CHUNK:(c + 1) * CHUNK, :]
                bsrc = bass.AP(tensor=src_ap.tensor, offset=src_ap.offset,
                               ap=[[0, 128]] + [list(d) for d in src_ap.ap])
                nc.sync.dma_start(out=kb, in_=bsrc)
                embed_chunk(c, xc, 4, yl)
                embed_chunk(c, xc, 5, y1l)
                xcs[c], kbs[c] = xc, kb

            hm_wave = {}
            for w in range(STEPS + 2):
                want = min(w // CHUNK + 1 + LEAD, NCHUNK)
                while next_chunk[0] < want:
                    load_chunk(next_chunk[0])
                    next_chunk[0] += 1
                new_hm = {}
                for l in range(3):
                    k = w - l
                    if not (0 <= k < STEPS):
                        continue
                    c, il = divmod(k, CHUNK)
                    x_i = xcs[c][:, il, :] if l == 0 else None
                    kb_i = kbs[c][:, il, :]
                    lin = hm_wave.get(l - 1)
                    hm = cell(l, k, x_i, kb_i, lin)
                    new_hm[l] = hm
                    if l == 2 and k >= WARM:
                        heads(k, hm)
                hm_wave = new_hm
                # release chunks no longer referenced (layer2 done past them)
                for c in [c for c in xcs if (c + 1) * CHUNK + 2 <= w + 1]:
                    xcs.pop(c), kbs.pop(c)

    nc.compile()
    return nc


def prepare_inputs(inp):
    """Host-side prep: returns (b_pi, in_maps) for the 8 cores."""
    f32 = np.float32
    rew = np.asarray(inp["rew"], f32)
    done = np.asarray(inp["done"], f32)
    gamma = float(np.asarray(inp["gamma"]).reshape(-1)[0])
    prob = np.asarray(inp["prob"], f32)
    y = np.asarray(inp["y"], f32)
    y1 = np.asarray(inp["y1"], f32)

    j_idx = np.arange(NSEG)[:, None]          # [16,1]
    i_idx = np.arange(STEPS)[None, :]         # [1,96]
    s_grid = SEG * j_idx - WARM + i_idx       # [16,96]
    valid = (s_grid >= 0)
    t_grid = np.clip(T - 1 - s_grid, 0, T - 1)  # [16,96]

    vmask = valid.astype(f32)                 # [16,96]

    def gather_feat(arr):
        # arr [B, T] -> [B, NSEG, STEPS] gathered, zeroed at pads
        return arr[:, t_grid] * vmask[None]

    rew_g = gather_feat(rew)
    done_g = gather_feat(done)
    prob_g = gather_feat(prob)

    # reset mask: done at that token; forced at the first real token of j=0
    reset = done_g.copy()                     # [B,16,96]
    reset[:, 0, WARM] = 1.0
    keep = 1.0 - reset                        # [B,16,96]
    # mask applied at END of step i must be the keep of step i+1
    keep_next = np.ones_like(keep)
    keep_next[:, :, :-1] = keep[:, :, 1:]

    # embed inputs gathered to lane order  [B,16,96,30]
    y_g = y[:, t_grid, :] * vmask[None, :, :, None]
    y1_g = y1[:, t_grid, :] * vmask[None, :, :, None]

    wih0_aug = np.zeros((7, 768), f32)
    wih0_aug[0:6] = np.asarray(inp["w_ih0"], f32).T
    brz = (np.asarray(inp["b_ih0"], f32) + np.asarray(inp["b_hh0"], f32))
    wih0_aug[6, 0:512] = brz[0:512]
    wih0_aug[6, 512:768] = np.asarray(inp["b_ih0"], f32)[512:768]

    def wT3(name):
        return np.ascontiguousarray(
            np.asarray(inp[name], f32).T.reshape(2, 128, 768))

    wfc_np = np.concatenate(
        [np.asarray(inp["w_fc_y"], f32), np.asarray(inp["w_fc_pi"], f32)], 0
    ).T.reshape(2, 128, 31)
    wfc_np = np.ascontiguousarray(wfc_np)

    we1_aug = np.zeros((31, 16), f32)
    we1_aug[0:30] = np.asarray(inp["w_e1"], f32).T
    we1_aug[30] = np.asarray(inp["b_e1"], f32)
    we2_aug = np.zeros((17, 1), f32)
    we2_aug[0:16] = np.asarray(inp["w_e2"], f32).T
    we2_aug[16, 0] = float(np.asarray(inp["b_e2"]).reshape(-1)[0])

    bias_np = np.zeros((128, NBIAS), f32)
    for l in (1, 2):
        comb = np.asarray(inp[f"b_ih{l}"], f32) + np.asarray(inp[f"b_hh{l}"], f32)
        for g in range(2):
            for mc in range(2):
                lo = g * 256 + mc * 128
                bias_np[:, _rzcol(l, g, mc)] = comb[lo:lo + 128]
        for mc in range(2):
            bias_np[:, _incol(l, mc)] = \
                np.asarray(inp[f"b_ih{l}"], f32)[512 + mc * 128:640 + mc * 128]
    for l in range(3):
        for mc in range(2):
            bias_np[:, _hncol(l, mc)] = \
                np.asarray(inp[f"b_hh{l}"], f32)[512 + mc * 128:640 + mc * 128]
    bias_np[0:30, BFCY_COL] = np.asarray(inp["b_fc_y"], f32)

    b_pi = float(np.asarray(inp["b_fc_pi"]).reshape(-1)[0])

    f16 = np.float16
    shared = dict(
        wih0=wih0_aug.astype(f16), wih1=wT3("w_ih1").astype(f16),
        wih2=wT3("w_ih2").astype(f16),
        whh0=wT3("w_hh0").astype(f16), whh1=wT3("w_hh1").astype(f16),
        whh2=wT3("w_hh2").astype(f16),
        wfc=wfc_np.astype(f16), we1=we1_aug, we2=we2_aug, bias=bias_np,
        zz=np.zeros((2, LANES), f16), o1=np.ones((CTOK,), f32),
    )

    in_maps = []
    for c in range(NCORES):
        R = slice(c * ROWS, (c + 1) * ROWS)
        # xf [7, STEPS, LANES]; lane = r*NSEG + j
        xf_np = np.zeros((7, STEPS, LANES), f32)

        def lanes(a):  # [16r, 16j, 96i] -> [96i, 256lane]
            return a.transpose(2, 0, 1).reshape(STEPS, LANES)

        xf_np[0] = lanes(rew_g[R])
        xf_np[1] = lanes(done_g[R])
        xf_np[2] = gamma * np.tile(vmask.T.reshape(STEPS, 1, NSEG),
                                   (1, ROWS, 1)).reshape(STEPS, LANES)
        xf_np[3] = lanes(prob_g[R])
        xf_np[6] = 1.0
        kp_np = lanes(keep_next[R])           # [96, 256]
        # y lanes [31, TOK]: token index = i*LANES + lane
        yl_np = np.ones((31, TOK), f32)
        yl_np[0:30] = y_g[R].transpose(3, 2, 0, 1).reshape(30, TOK)
        y1l_np = np.ones((31, TOK), f32)
        y1l_np[0:30] = y1_g[R].transpose(3, 2, 0, 1).reshape(30, TOK)
        in_maps.append(dict(shared, xf=xf_np.astype(np.float16),
                            kp=kp_np.astype(np.float16), yl=yl_np, y1l=y1l_np))
    return b_pi, in_maps


def assemble_outputs(results):
    pi_proc = np.zeros((B, T), np.float32)
    yo_proc = np.zeros((B, T, YD), np.float32)
    for c in range(NCORES):
        R = slice(c * ROWS, (c + 1) * ROWS)
        yo_c = results[c]["yo"]               # [64, 30, 256]
        pi_c = results[c]["pi"]               # [64, 256]
        yo_proc[R] = yo_c.reshape(CSTEPS, YD, ROWS, NSEG) \
            .transpose(2, 3, 0, 1).reshape(ROWS, T, YD)
        pi_proc[R] = pi_c.reshape(CSTEPS, ROWS, NSEG) \
            .transpose(1, 2, 0).reshape(ROWS, T)
    # processing order s -> original time t = T-1-s
    return np.ascontiguousarray(pi_proc[:, ::-1]), \
        np.ascontiguousarray(yo_proc[:, ::-1])


def kernel(**inputs):
    b_pi, in_maps = prepare_inputs(inputs)
    nc = build_nc(b_pi)
    res = run_bass_kernel_spmd(nc, in_maps, list(range(NCORES)))
    return assemble_outputs(res.results)
